# revision 35
# baseline (speedup 1.0000x reference)
"""Trainium2 Bass kernel for nn_MultiHeadedAttention (B=4, L=2048, D=1024, no head split).

Returns (result, atte) exactly like the reference:
  q/k/v = x @ W{q,k,v}.T + b;  scores = q k^T / sqrt(D);  atte = softmax(scores);
  out = atte @ v;  result = swapaxes(out,1,2).reshape(B, -1, D)

Sharding: 8 cores = (batch b, query-half h).  Each core computes K/V for its
full batch-b sequence (duplicated across the pair) and Q/scores/atte/out for
its 1024 queries.  The key sequence is rotated per-core so the core's query
block is always tokens [0,1024) of its local sequence — this keeps the SPMD
program identical across cores; the host un-permutes atte columns afterwards.

Numerics: matmuls run in float32r (PE "transpose mode" fp32: 4x faster than
plain fp32, ~9 significant mantissa bits on operands, fp32 accumulation).
The softmax itself (exp, sums, normalization) and the atte output stay fp32.
v-bias is folded out on the host:  atte @ (v+bv) == atte @ v + bv  because
softmax rows sum to 1.
"""

import os
import sys

for _p in ("/opt/trn_rl_repo", "/root/.axon_site/_ro/trn_rl_repo"):
    if os.path.isdir(_p) and _p not in sys.path:
        sys.path.append(_p)

import numpy as np

B, L, D = 4, 2048, 1024
NCORES = 8
P = 128
ED = D // P          # 8 subtiles along d/e
TT = L // P          # 16 token subtiles
QL = L // 2          # queries per core
NQ = QL // P         # 8 query tiles per core
KSLAB = 256          # scores k-slab width
NSLAB = L // KSLAB   # 8
TCH = 256            # projection token-chunk width
NTCH = L // TCH      # 8
NFREE = 512          # max fp32 moving free dim
SOFTMAX_SCALE = 1.0 / 32.0   # 1/sqrt(D)

USE_F32R = True      # float32r: 4x faster fp32 matmul mode


def ts(i, size):
    return slice(i * size, (i + 1) * size)


_BUILT = None


def build_bass():
    """Build the single-core SPMD Bass program. Returns nc."""
    global _BUILT
    if _BUILT is not None:
        return _BUILT

    import concourse.bass as bass
    import concourse.bacc as bacc
    import concourse.tile as tile
    from concourse import mybir
    from concourse.masks import make_identity

    f32 = mybir.dt.float32
    fmm = mybir.dt.float32r if USE_F32R else f32

    nc = bacc.Bacc(None)

    # x^T in chunk-major partition-contiguous layout:
    #   xt[c, p, dd, t'] = x_local[c*TCH + t', dd*128 + p]
    # so each chunk load is one clean 2D DMA (8KB contiguous per partition).
    xt_h = nc.declare_dram_parameter("xt", [NTCH, P, ED, TCH], f32, isOutput=False)
    wqt_h = nc.declare_dram_parameter("wqt", [ED, P, D], f32, isOutput=False)
    wkt_h = nc.declare_dram_parameter("wkt", [ED, P, D], f32, isOutput=False)
    wvt_h = nc.declare_dram_parameter("wvt", [ED, P, D], f32, isOutput=False)
    bq_h = nc.declare_dram_parameter("bq2", [P, ED], f32, isOutput=False)
    bk_h = nc.declare_dram_parameter("bk2", [P, ED], f32, isOutput=False)
    atte_h = nc.declare_dram_parameter("atte", [QL, L], f32, isOutput=True)
    out_h = nc.declare_dram_parameter("out", [QL, D], f32, isOutput=True)

    with tile.TileContext(nc) as tc:
        with (
            tc.tile_pool(name="persist", bufs=1) as persist,
            tc.tile_pool(name="resident", bufs=1) as resident,
            tc.tile_pool(name="ktdram", bufs=1, space="DRAM") as ktdram,
        ):
            ident = persist.tile([P, P], f32)
            make_identity(nc, ident)

            bq_sb = persist.tile([P, ED], f32)
            bk_sb = persist.tile([P, ED], f32)
            nc.sync.dma_start(out=bq_sb[:], in_=bq_h[:])
            nc.sync.dma_start(out=bk_sb[:], in_=bk_h[:])

            # V resident:  V[p, tt, e] = v[tt*128+p, e]   (f32r)
            v_sb = resident.tile([P, TT, D], fmm)
            # Q^T resident: QT[p, ee, t] = q[t, ee*128+p]  (f32r)
            qt_sb = resident.tile([P, ED, QL], fmm)
            # softmax partial sums / reciprocals
            sums_sb = persist.tile([P, NQ, NSLAB], f32)
            tot_sb = persist.tile([P, NQ], f32)
            recip_sb = persist.tile([P, NQ], f32)

            # K^T spill space in DRAM (f32r bytes), slab-major so each
            # scores-phase slab reload is one partition-contiguous DMA
            kt_dram = ktdram.tile([NTCH, P, ED, TCH], fmm)

            # ---------------- projection phase ----------------
            # All three W's stay resident in f32r; x streams per phase in
            # 256-token chunks (cast fp32 -> f32r on the fly).  Each chunk
            # carries ~7.5us of PE work vs ~2.8us of DMA, so PE stays fed.
            with (
                tc.tile_pool(name="wp", bufs=1) as wp,
                tc.tile_pool(name="wraw", bufs=2) as wraw,
                tc.tile_pool(name="xcp", bufs=3) as xcp,
                tc.tile_pool(name="xrp", bufs=2) as xrp,
                tc.tile_pool(name="kout", bufs=2) as kout,
                tc.tile_pool(name="ppsum", bufs=6, space="PSUM") as ppsum,
            ):
                def load_w(wh, tag):
                    WCH = 512
                    w_sb = wp.tile([P, ED, D], fmm, tag=tag)
                    for dd in range(ED):
                        for hh in range(D // WCH):
                            wr = wraw.tile([P, WCH], f32, tag="wr")
                            nc.sync.dma_start(out=wr[:], in_=wh[dd, :, ts(hh, WCH)])
                            nc.vector.tensor_copy(w_sb[:, dd, ts(hh, WCH)], wr[:])
                    return w_sb

                wk_sb = load_w(wkt_h, "wk")
                wv_sb = load_w(wvt_h, "wv")

                def x_chunk(tch, engine_idx):
                    """Stream one 256-token chunk of x^T, round to f32r."""
                    xr = xrp.tile([P, ED, TCH], f32, tag="xr")
                    nc.sync.dma_start(out=xr[:], in_=xt_h[tch])
                    xc = xcp.tile([P, ED, TCH], fmm, tag="xc")
                    if engine_idx % 2 == 0:
                        nc.scalar.copy(xc[:], xr[:])
                    else:
                        nc.vector.tensor_copy(xc[:], xr[:])
                    return xc

                # --- K^T = Wk x^T  -> spill to DRAM ---
                for tch in range(NTCH):
                    xc = x_chunk(tch, tch)
                    for ee in range(ED):
                        ps = ppsum.tile([P, TCH], f32, tag="pp")
                        for dd in range(ED):
                            nc.tensor.matmul(
                                ps[:],
                                wk_sb[:, dd, ts(ee, P)],
                                xc[:, dd, :],
                                start=(dd == 0),
                                stop=(dd == ED - 1),
                            )
                        ko = kout.tile([P, TCH], fmm, tag="ko")
                        nc.vector.tensor_scalar_add(ko[:], ps[:], bk_sb[:, ee : ee + 1])
                        nc.sync.dma_start(out=kt_dram[tch, :, ee, :], in_=ko[:])

                # --- V = x Wv^T  -> resident (bias folded out on host) ---
                # Wq reuses Wk's slot (K phase is done); loads during V phase
                wq_sb = load_w(wqt_h, "wk")
                for tch in range(NTCH):
                    xc = x_chunk(tch, tch)
                    for tl in range(TCH // P):
                        tt = tch * (TCH // P) + tl
                        for ec in range(D // NFREE):
                            ps = ppsum.tile([P, NFREE], f32, tag="pp")
                            for dd in range(ED):
                                nc.tensor.matmul(
                                    ps[:],
                                    xc[:, dd, ts(tl, P)],
                                    wv_sb[:, dd, ts(ec, NFREE)],
                                    start=(dd == 0),
                                    stop=(dd == ED - 1),
                                )
                            nc.vector.tensor_copy(v_sb[:, tt, ts(ec, NFREE)], ps[:])

                # --- Q^T = Wq x^T (first QL tokens) -> resident ---
                for tch in range(QL // TCH):
                    xc = x_chunk(tch, tch)
                    for ee in range(ED):
                        ps = ppsum.tile([P, TCH], f32, tag="pp")
                        for dd in range(ED):
                            nc.tensor.matmul(
                                ps[:],
                                wq_sb[:, dd, ts(ee, P)],
                                xc[:, dd, :],
                                start=(dd == 0),
                                stop=(dd == ED - 1),
                            )
                        nc.vector.tensor_scalar_add(
                            qt_sb[:, ee, ts(tch, TCH)], ps[:], bq_sb[:, ee : ee + 1]
                        )

            # ---------------- attention phase ----------------
            with (
                tc.tile_pool(name="ep", bufs=1) as ep,
                tc.tile_pool(name="ktp", bufs=2) as ktp,
                tc.tile_pool(name="etp", bufs=2) as etp,
                tc.tile_pool(name="osb", bufs=2) as osb,
                tc.tile_pool(name="spsum", bufs=4, space="PSUM") as spsum,
                tc.tile_pool(name="tpsum", bufs=2, space="PSUM") as tpsum,
                tc.tile_pool(name="apsum", bufs=2, space="PSUM") as apsum,
            ):
                e_sb = ep.tile([P, NQ, L], f32)

                # scores + exp, k-slab major (K^T streamed once from DRAM)
                for ks in range(NSLAB):
                    kt_t = ktp.tile([P, ED, KSLAB], fmm, tag="kt")
                    nc.sync.dma_start(out=kt_t[:], in_=kt_dram[ks])
                    for qq in range(NQ):
                        ps = spsum.tile([P, KSLAB], f32, tag="sp")
                        for ee in range(ED):
                            nc.tensor.matmul(
                                ps[:],
                                qt_sb[:, ee, ts(qq, P)],
                                kt_t[:, ee, :],
                                start=(ee == 0),
                                stop=(ee == ED - 1),
                            )
                        nc.scalar.activation(
                            e_sb[:, qq, ts(ks, KSLAB)],
                            ps[:],
                            mybir.ActivationFunctionType.Exp,
                            scale=SOFTMAX_SCALE,
                            accum_out=sums_sb[:, qq, ks : ks + 1],
                        )

                for qq in range(NQ):
                    # row sums -> reciprocal (normalization deferred to evictions)
                    nc.vector.tensor_reduce(
                        tot_sb[:, qq : qq + 1],
                        sums_sb[:, qq, :],
                        axis=mybir.AxisListType.X,
                        op=mybir.AluOpType.add,
                    )
                    nc.vector.reciprocal(recip_sb[:, qq : qq + 1], tot_sb[:, qq : qq + 1])

                    # transpose UNnormalized exp for the A @ V matmul
                    et_t = etp.tile([P, TT, P], fmm, tag="et")
                    for kk in range(TT):
                        pt = tpsum.tile([P, P], f32, tag="tp")
                        nc.tensor.transpose(pt[:], e_sb[:, qq, ts(kk, P)], ident[:])
                        nc.scalar.copy(et_t[:, kk, :], pt[:])

                    # out = (E @ V) * recip  (1/rowsum folded into eviction)
                    ot = osb.tile([P, D], f32, tag="ot")
                    for dc in range(D // NFREE):
                        pa = apsum.tile([P, NFREE], f32, tag="ap")
                        for kk in range(TT):
                            nc.tensor.matmul(
                                pa[:],
                                et_t[:, kk, :],
                                v_sb[:, kk, ts(dc, NFREE)],
                                start=(kk == 0),
                                stop=(kk == TT - 1),
                            )
                        nc.scalar.activation(
                            ot[:, ts(dc, NFREE)],
                            pa[:],
                            mybir.ActivationFunctionType.Copy,
                            scale=recip_sb[:, qq : qq + 1],
                        )
                    nc.scalar.dma_start(out=out_h[ts(qq, P), :], in_=ot[:])

                    # normalize atte in place (after the transposes read E) + store
                    nc.vector.tensor_scalar_mul(
                        e_sb[:, qq, :], e_sb[:, qq, :], recip_sb[:, qq : qq + 1]
                    )
                    nc.scalar.dma_start(out=atte_h[ts(qq, P), :], in_=e_sb[:, qq, :])

    nc.finalize()
    _BUILT = nc
    return nc


def make_in_maps(x, Wq, bq, Wk, bk, Wv, bv):
    """Per-core input dicts. Core c handles batch c//2, query-half c%2."""
    x = np.ascontiguousarray(np.asarray(x, dtype=np.float32))
    wqt = np.ascontiguousarray(np.asarray(Wq, np.float32).T).reshape(ED, P, D)
    wkt = np.ascontiguousarray(np.asarray(Wk, np.float32).T).reshape(ED, P, D)
    wvt = np.ascontiguousarray(np.asarray(Wv, np.float32).T).reshape(ED, P, D)
    bq2 = np.ascontiguousarray(np.asarray(bq, np.float32).reshape(ED, P).T)
    bk2 = np.ascontiguousarray(np.asarray(bk, np.float32).reshape(ED, P).T)

    in_maps = []
    for c in range(NCORES):
        b, h = divmod(c, 2)
        if h == 0:
            xl = x[b]
        else:
            xl = np.concatenate([x[b, QL:], x[b, :QL]], axis=0)
        # [NTCH, P, ED, TCH]: xt[c, p, dd, t'] = xl[c*TCH + t', dd*128 + p]
        xt = np.ascontiguousarray(
            xl.reshape(NTCH, TCH, ED, P).transpose(0, 3, 2, 1)
        )
        in_maps.append(
            {"xt": xt, "wqt": wqt, "wkt": wkt, "wvt": wvt, "bq2": bq2, "bk2": bk2}
        )
    return in_maps


def assemble(results, bv, dtype=np.float32):
    """Gather per-core (atte, out) into full (result, atte)."""
    bv = np.asarray(bv, dtype)
    atte = np.empty((B, L, L), dtype=dtype)
    outf = np.empty((B, L, D), dtype=dtype)
    for c in range(NCORES):
        b, h = divmod(c, 2)
        qo = h * QL
        a = results[c]["atte"]
        o = results[c]["out"]
        outf[b, qo : qo + QL, :] = o + bv  # softmax rows sum to 1
        # columns were computed in rotated key order: local j -> global (qo + j) mod L
        atte[b, qo : qo + QL, qo:] = a[:, : L - qo]
        atte[b, qo : qo + QL, :qo] = a[:, L - qo :]
    result = np.swapaxes(outf, 1, 2).reshape(B, -1, D)
    return result, atte


def run_on_hw(in_maps, trace=False, **kw):
    from concourse.bass_utils import run_bass_kernel_spmd

    nc = build_bass()
    return run_bass_kernel_spmd(nc, in_maps, list(range(NCORES)), trace=trace, **kw)


def kernel(x, Wq, bq, Wk, bk, Wv, bv):
    in_maps = make_in_maps(x, Wq, bq, Wk, bk, Wv, bv)
    res = run_on_hw(in_maps, trace=False)
    return assemble(res.results, bv)


# revision 36
# speedup vs baseline: 1.0596x; 1.0596x over previous
"""Trainium2 Bass kernel for nn_MultiHeadedAttention (B=4, L=2048, D=1024, no head split).

Returns (result, atte) exactly like the reference:
  q/k/v = x @ W{q,k,v}.T + b;  scores = q k^T / sqrt(D);  atte = softmax(scores);
  out = atte @ v;  result = swapaxes(out,1,2).reshape(B, -1, D)

Sharding: 8 cores = (batch b, query-half h).  Each core computes K/V for its
full batch-b sequence (duplicated across the pair) and Q/scores/atte/out for
its 1024 queries.  The key sequence is rotated per-core so the core's query
block is always tokens [0,1024) of its local sequence — this keeps the SPMD
program identical across cores; the host un-permutes atte columns afterwards.

Numerics: matmuls run in float32r (PE "transpose mode" fp32: 4x faster than
plain fp32, ~9 significant mantissa bits on operands, fp32 accumulation).
The softmax itself (exp, sums, normalization) and the atte output stay fp32.
v-bias is folded out on the host:  atte @ (v+bv) == atte @ v + bv  because
softmax rows sum to 1.
"""

import os
import sys

for _p in ("/opt/trn_rl_repo", "/root/.axon_site/_ro/trn_rl_repo"):
    if os.path.isdir(_p) and _p not in sys.path:
        sys.path.append(_p)

import numpy as np

B, L, D = 4, 2048, 1024
NCORES = 8
P = 128
ED = D // P          # 8 subtiles along d/e
TT = L // P          # 16 token subtiles
QL = L // 2          # queries per core
NQ = QL // P         # 8 query tiles per core
KSLAB = 256          # scores k-slab width
NSLAB = L // KSLAB   # 8
TCH = 256            # projection token-chunk width
NTCH = L // TCH      # 8
NFREE = 512          # max fp32 moving free dim
SOFTMAX_SCALE = 1.0 / 32.0   # 1/sqrt(D)

USE_F32R = True      # float32r: 4x faster fp32 matmul mode


def ts(i, size):
    return slice(i * size, (i + 1) * size)


_BUILT = None


def build_bass():
    """Build the single-core SPMD Bass program. Returns nc."""
    global _BUILT
    if _BUILT is not None:
        return _BUILT

    import concourse.bass as bass
    import concourse.bacc as bacc
    import concourse.tile as tile
    from concourse import mybir
    from concourse.masks import make_identity

    f32 = mybir.dt.float32
    fmm = mybir.dt.float32r if USE_F32R else f32

    nc = bacc.Bacc(None)

    # x^T in chunk-major partition-contiguous layout:
    #   xt[c, p, dd, t'] = x_local[c*TCH + t', dd*128 + p]
    # so each chunk load is one clean 2D DMA (8KB contiguous per partition).
    xt_h = nc.declare_dram_parameter("xt", [NTCH, P, ED, TCH], f32, isOutput=False)
    wqt_h = nc.declare_dram_parameter("wqt", [ED, P, D], f32, isOutput=False)
    wkt_h = nc.declare_dram_parameter("wkt", [ED, P, D], f32, isOutput=False)
    wvt_h = nc.declare_dram_parameter("wvt", [ED, P, D], f32, isOutput=False)
    bq_h = nc.declare_dram_parameter("bq2", [P, ED], f32, isOutput=False)
    bk_h = nc.declare_dram_parameter("bk2", [P, ED], f32, isOutput=False)
    atte_h = nc.declare_dram_parameter("atte", [QL, L], f32, isOutput=True)
    out_h = nc.declare_dram_parameter("out", [QL, D], f32, isOutput=True)

    with tile.TileContext(nc) as tc:
        with (
            tc.tile_pool(name="persist", bufs=1) as persist,
            tc.tile_pool(name="resident", bufs=1) as resident,
            tc.tile_pool(name="ktdram", bufs=1, space="DRAM") as ktdram,
        ):
            ident = persist.tile([P, P], f32)
            make_identity(nc, ident)

            bq_sb = persist.tile([P, ED], f32)
            bk_sb = persist.tile([P, ED], f32)
            nc.sync.dma_start(out=bq_sb[:], in_=bq_h[:])
            nc.sync.dma_start(out=bk_sb[:], in_=bk_h[:])

            # V resident:  V[p, tt, e] = v[tt*128+p, e]   (f32r)
            v_sb = resident.tile([P, TT, D], fmm)
            # Q^T resident: QT[p, ee, t] = q[t, ee*128+p]  (f32r)
            qt_sb = resident.tile([P, ED, QL], fmm)
            # softmax partial sums / reciprocals
            sums_sb = persist.tile([P, NQ, NSLAB], f32)
            tot_sb = persist.tile([P, NQ], f32)
            recip_sb = persist.tile([P, NQ], f32)

            # K^T spill space in DRAM (f32r bytes), slab-major so each
            # scores-phase slab reload is one partition-contiguous DMA
            kt_dram = ktdram.tile([NTCH, P, ED, TCH], fmm)

            # ---------------- projection phase ----------------
            # All three W's stay resident in f32r; x streams per phase in
            # 256-token chunks (cast fp32 -> f32r on the fly).  Each chunk
            # carries ~7.5us of PE work vs ~2.8us of DMA, so PE stays fed.
            with (
                tc.tile_pool(name="wp", bufs=1) as wp,
                tc.tile_pool(name="wraw", bufs=2) as wraw,
                tc.tile_pool(name="xcp", bufs=2) as xcp,
                tc.tile_pool(name="xrp", bufs=2) as xrp,
                tc.tile_pool(name="kout", bufs=2) as kout,
                tc.tile_pool(name="ppsum", bufs=6, space="PSUM") as ppsum,
            ):
                def load_w(wh, tag):
                    w_sb = wp.tile([P, ED, D], fmm, tag=tag)
                    for dd in range(ED):
                        wr = wraw.tile([P, D], f32, tag="wr")
                        nc.sync.dma_start(out=wr[:], in_=wh[dd])
                        nc.vector.tensor_copy(w_sb[:, dd, :], wr[:])
                    return w_sb

                wk_sb = load_w(wkt_h, "wk")
                wv_sb = load_w(wvt_h, "wv")

                def x_chunk(tch, engine_idx):
                    """Stream one 256-token chunk of x^T, round to f32r."""
                    xr = xrp.tile([P, ED, TCH], f32, tag="xr")
                    nc.sync.dma_start(out=xr[:], in_=xt_h[tch])
                    xc = xcp.tile([P, ED, TCH], fmm, tag="xc")
                    nc.scalar.copy(xc[:], xr[:])
                    return xc

                # --- K^T = Wk x^T  -> spill to DRAM ---
                for tch in range(NTCH):
                    xc = x_chunk(tch, tch)
                    for ee in range(ED):
                        ps = ppsum.tile([P, TCH], f32, tag="pp")
                        for dd in range(ED):
                            nc.tensor.matmul(
                                ps[:],
                                wk_sb[:, dd, ts(ee, P)],
                                xc[:, dd, :],
                                start=(dd == 0),
                                stop=(dd == ED - 1),
                            )
                        ko = kout.tile([P, TCH], fmm, tag="ko")
                        nc.vector.tensor_scalar_add(ko[:], ps[:], bk_sb[:, ee : ee + 1])
                        nc.sync.dma_start(out=kt_dram[tch, :, ee, :], in_=ko[:])

                # --- V = x Wv^T  -> resident (bias folded out on host) ---
                # Wq reuses Wk's slot (K phase is done); loads during V phase
                wq_sb = load_w(wqt_h, "wk")
                for tch in range(NTCH):
                    xc = x_chunk(tch, tch)
                    for tl in range(TCH // P):
                        tt = tch * (TCH // P) + tl
                        for ec in range(D // NFREE):
                            ps = ppsum.tile([P, NFREE], f32, tag="pp")
                            for dd in range(ED):
                                nc.tensor.matmul(
                                    ps[:],
                                    xc[:, dd, ts(tl, P)],
                                    wv_sb[:, dd, ts(ec, NFREE)],
                                    start=(dd == 0),
                                    stop=(dd == ED - 1),
                                )
                            nc.vector.tensor_copy(v_sb[:, tt, ts(ec, NFREE)], ps[:])

                # --- Q^T = Wq x^T (first QL tokens) -> resident ---
                for tch in range(QL // TCH):
                    xc = x_chunk(tch, tch)
                    for ee in range(ED):
                        ps = ppsum.tile([P, TCH], f32, tag="pp")
                        for dd in range(ED):
                            nc.tensor.matmul(
                                ps[:],
                                wq_sb[:, dd, ts(ee, P)],
                                xc[:, dd, :],
                                start=(dd == 0),
                                stop=(dd == ED - 1),
                            )
                        nc.vector.tensor_scalar_add(
                            qt_sb[:, ee, ts(tch, TCH)], ps[:], bq_sb[:, ee : ee + 1]
                        )

            # ---------------- attention phase ----------------
            with (
                tc.tile_pool(name="ep", bufs=1) as ep,
                tc.tile_pool(name="ktp", bufs=2) as ktp,
                tc.tile_pool(name="etp", bufs=2) as etp,
                tc.tile_pool(name="osb", bufs=2) as osb,
                tc.tile_pool(name="spsum", bufs=4, space="PSUM") as spsum,
                tc.tile_pool(name="tpsum", bufs=2, space="PSUM") as tpsum,
                tc.tile_pool(name="apsum", bufs=2, space="PSUM") as apsum,
            ):
                e_sb = ep.tile([P, NQ, L], f32)

                # scores + exp, k-slab major (K^T streamed once from DRAM)
                for ks in range(NSLAB):
                    kt_t = ktp.tile([P, ED, KSLAB], fmm, tag="kt")
                    nc.sync.dma_start(out=kt_t[:], in_=kt_dram[ks])
                    for qq in range(NQ):
                        ps = spsum.tile([P, KSLAB], f32, tag="sp")
                        for ee in range(ED):
                            nc.tensor.matmul(
                                ps[:],
                                qt_sb[:, ee, ts(qq, P)],
                                kt_t[:, ee, :],
                                start=(ee == 0),
                                stop=(ee == ED - 1),
                            )
                        nc.scalar.activation(
                            e_sb[:, qq, ts(ks, KSLAB)],
                            ps[:],
                            mybir.ActivationFunctionType.Exp,
                            scale=SOFTMAX_SCALE,
                            accum_out=sums_sb[:, qq, ks : ks + 1],
                        )

                for qq in range(NQ):
                    # row sums -> reciprocal (normalization deferred to evictions)
                    nc.vector.tensor_reduce(
                        tot_sb[:, qq : qq + 1],
                        sums_sb[:, qq, :],
                        axis=mybir.AxisListType.X,
                        op=mybir.AluOpType.add,
                    )
                    nc.vector.reciprocal(recip_sb[:, qq : qq + 1], tot_sb[:, qq : qq + 1])

                    # transpose UNnormalized exp for the A @ V matmul
                    et_t = etp.tile([P, TT, P], fmm, tag="et")
                    for kk in range(TT):
                        pt = tpsum.tile([P, P], f32, tag="tp")
                        nc.tensor.transpose(pt[:], e_sb[:, qq, ts(kk, P)], ident[:])
                        nc.scalar.copy(et_t[:, kk, :], pt[:])

                    # out = (E @ V) * recip  (1/rowsum folded into eviction)
                    ot = osb.tile([P, D], f32, tag="ot")
                    for dc in range(D // NFREE):
                        pa = apsum.tile([P, NFREE], f32, tag="ap")
                        for kk in range(TT):
                            nc.tensor.matmul(
                                pa[:],
                                et_t[:, kk, :],
                                v_sb[:, kk, ts(dc, NFREE)],
                                start=(kk == 0),
                                stop=(kk == TT - 1),
                            )
                        nc.scalar.activation(
                            ot[:, ts(dc, NFREE)],
                            pa[:],
                            mybir.ActivationFunctionType.Copy,
                            scale=recip_sb[:, qq : qq + 1],
                        )
                    nc.scalar.dma_start(out=out_h[ts(qq, P), :], in_=ot[:])

                    # normalize atte in place (after the transposes read E) + store
                    nc.vector.tensor_scalar_mul(
                        e_sb[:, qq, :], e_sb[:, qq, :], recip_sb[:, qq : qq + 1]
                    )
                    nc.scalar.dma_start(out=atte_h[ts(qq, P), :], in_=e_sb[:, qq, :])

    nc.finalize()
    _BUILT = nc
    return nc


def make_in_maps(x, Wq, bq, Wk, bk, Wv, bv):
    """Per-core input dicts. Core c handles batch c//2, query-half c%2."""
    x = np.ascontiguousarray(np.asarray(x, dtype=np.float32))
    wqt = np.ascontiguousarray(np.asarray(Wq, np.float32).T).reshape(ED, P, D)
    wkt = np.ascontiguousarray(np.asarray(Wk, np.float32).T).reshape(ED, P, D)
    wvt = np.ascontiguousarray(np.asarray(Wv, np.float32).T).reshape(ED, P, D)
    bq2 = np.ascontiguousarray(np.asarray(bq, np.float32).reshape(ED, P).T)
    bk2 = np.ascontiguousarray(np.asarray(bk, np.float32).reshape(ED, P).T)

    in_maps = []
    for c in range(NCORES):
        b, h = divmod(c, 2)
        if h == 0:
            xl = x[b]
        else:
            xl = np.concatenate([x[b, QL:], x[b, :QL]], axis=0)
        # [NTCH, P, ED, TCH]: xt[c, p, dd, t'] = xl[c*TCH + t', dd*128 + p]
        xt = np.ascontiguousarray(
            xl.reshape(NTCH, TCH, ED, P).transpose(0, 3, 2, 1)
        )
        in_maps.append(
            {"xt": xt, "wqt": wqt, "wkt": wkt, "wvt": wvt, "bq2": bq2, "bk2": bk2}
        )
    return in_maps


def assemble(results, bv, dtype=np.float32):
    """Gather per-core (atte, out) into full (result, atte)."""
    bv = np.asarray(bv, dtype)
    atte = np.empty((B, L, L), dtype=dtype)
    outf = np.empty((B, L, D), dtype=dtype)
    for c in range(NCORES):
        b, h = divmod(c, 2)
        qo = h * QL
        a = results[c]["atte"]
        o = results[c]["out"]
        outf[b, qo : qo + QL, :] = o + bv  # softmax rows sum to 1
        # columns were computed in rotated key order: local j -> global (qo + j) mod L
        atte[b, qo : qo + QL, qo:] = a[:, : L - qo]
        atte[b, qo : qo + QL, :qo] = a[:, L - qo :]
    result = np.swapaxes(outf, 1, 2).reshape(B, -1, D)
    return result, atte


def run_on_hw(in_maps, trace=False, **kw):
    from concourse.bass_utils import run_bass_kernel_spmd

    nc = build_bass()
    return run_bass_kernel_spmd(nc, in_maps, list(range(NCORES)), trace=trace, **kw)


def kernel(x, Wq, bq, Wk, bk, Wv, bv):
    in_maps = make_in_maps(x, Wq, bq, Wk, bk, Wv, bv)
    res = run_on_hw(in_maps, trace=False)
    return assemble(res.results, bv)
